# revision 32
# baseline (speedup 1.0000x reference)
"""Trainium2 Bass kernel for ExplicitRandomWalkEncoder, v4.

Math:
    x_encoded = x @ W_f.T + b_f;  feats = x_encoded[walks]
    h_T = GRU(feats)  (torch gate order r,z,n)
Fold: gx = x[walks] @ (W_ih @ W_f).T + (W_ih b_f + b_ih), so the device
gathers raw x rows and applies W_c = W_ih @ W_f.

Sharding: data-parallel over walks; 2048 walks/core, x + weights replicated.

Gather engine: x is converted to bf16 on the host (256-byte rows). The
gather is drain-limited (~2.1 ns/descriptor across 4 SWDGE queues), so the
schedule interleaves phase-1 (HBM->bucket, deduped per 2-step group) and
phase-2 (bucket->xT transpose un-bucket) at step-pair granularity:

  for each pair p: p1(group p) then p2(pair p)

so delivery (~17 us/pair) stays ahead of consumption (~20 us/pair) with no
multi-call phase-1 blobs. Phase-1 index padding uses -1 (trailing negative
indices generate no descriptors, so padding costs no drain bandwidth).

GRU step (hidden-major, 4 quarter-chunks of 512 walks):
  PE   : r|z psum = Wc_rz @ xT (bf16) + Whh_rz @ h;  nx, nh psum
  ACT  : r,z = sigmoid(rz + b) -> bf16; n = tanh(narg + b_xn) -> bf16
  DVE  : rhn = (nh + b_hn) * r; narg = rhn + nx;
         d = h - n; d *= z; h' = n + d
"""

import math

import ml_dtypes
import numpy as np

N_NODES = 200000
D = 128
H = 128
B_TOTAL = 16384
T = 20
NCORES = 8
B = B_TOTAL // NCORES      # 2048 walks per core
Q = 512                    # quarter chunk
HALF = 1024
NPAIR = T // 2

NREG = 7                   # region count: ceil(N_NODES/NREG) < 32768
P1_MAX = 1024              # max descriptors per HBM dma_gather
P2_MAX = 896               # max descriptors per SBUF-source dma_gather

_CACHE = {}


def _roundup(x, m):
    return (x + m - 1) // m * m


def _wrap16(ids):
    """flat int list (len % 16 == 0) -> [128, len/16] int16 tile layout."""
    a = np.asarray(ids, dtype=np.int16).reshape(-1, 16).T   # [16, n/16]
    return np.tile(a, (8, 1))


def _host_prep(x, walks, W_f, b_f, W_ih, W_hh, b_ih, b_hh):
    x = np.asarray(x, dtype=np.float32)
    walks = np.asarray(walks).astype(np.int64)
    W_f = np.asarray(W_f, dtype=np.float32)
    b_f = np.asarray(b_f, dtype=np.float32)
    W_ih = np.asarray(W_ih, dtype=np.float32)
    W_hh = np.asarray(W_hh, dtype=np.float32)
    b_ih = np.asarray(b_ih, dtype=np.float32)
    b_hh = np.asarray(b_hh, dtype=np.float32)

    xb = np.ascontiguousarray(x).astype(ml_dtypes.bfloat16)

    W_c = (W_ih @ W_f).astype(np.float32)
    b_c = (W_ih @ b_f + b_ih).astype(np.float32)
    wcb = np.ascontiguousarray(W_c.T).astype(ml_dtypes.bfloat16)  # [128, 384]
    whf = np.ascontiguousarray(W_hh.T).astype(np.float32)         # [128, 384]
    bias = np.zeros((128, 4), dtype=np.float32)
    bias[:, 0] = b_c[0:128] + b_hh[0:128]       # b_r
    bias[:, 1] = b_c[128:256] + b_hh[128:256]   # b_z
    bias[:, 2] = b_hh[256:384]                  # b_hn
    bias[:, 3] = b_c[256:384]                   # b_xn
    rz_same = bool(np.allclose(bias[:, 0], bias[:, 1]))
    bhn_zero = bool(np.allclose(bias[:, 2], 0.0))

    # ---- per-core, per-pair bucketing (group g = steps 2g, 2g+1)
    # nodes[c][t][i] = walks[c*B + i, t]
    nodes = walks.reshape(NCORES, B, T).transpose(0, 2, 1)  # [C, T, B]
    R = math.ceil(N_NODES / NREG)

    caps = []       # per group: tuple of per-region capblk (uniform across cores)
    for g in range(NPAIR):
        t0, t1 = 2 * g, 2 * g + 2
        mx = np.zeros(NREG, dtype=np.int64)
        for c in range(NCORES):
            uniq = np.unique(nodes[c, t0:t1, :].ravel())
            cnt = np.bincount(np.minimum(uniq // R, NREG - 1), minlength=NREG)
            mx = np.maximum(mx, cnt)
        capr = [_roundup(max(int(v), 128), 128) for v in mx]
        assert max(capr) <= 1024, f"bucket overflow risk: {capr}"
        caps.append(tuple(cc // 128 for cc in capr))   # blocks per region

    p1cols = max((sum(cb) * 128) // 16 for cb in caps)
    p2cols = (2 * B) // 16

    in_maps = []
    for c in range(NCORES):
        m = {"xb": xb, "wcb": wcb, "whf": whf, "bias": bias}
        for g, capblk in enumerate(caps):
            t0 = 2 * g
            base = np.cumsum([0] + [cb * 128 for cb in capblk])
            p1 = [np.zeros(cb * 128, dtype=np.int16) for cb in capblk]
            cnt = np.zeros(NREG, dtype=np.int64)
            pos = np.zeros((2, B), dtype=np.int64)
            nd = nodes[c, t0:t0 + 2, :]
            rg = np.minimum(nd // R, NREG - 1)
            loc = nd - rg * R
            slot = {}
            for s in range(2):
                for i in range(B):
                    n_ = nd[s, i]
                    k = slot.get(n_)
                    if k is None:
                        r = rg[s, i]
                        k = base[r] + cnt[r]
                        p1[r][cnt[r]] = loc[s, i]
                        cnt[r] += 1
                        slot[n_] = k
                    pos[s, i] = k
            assert pos.max() < 32768
            p1t = _wrap16(np.concatenate(p1))
            p2t = _wrap16(pos.ravel().astype(np.int16))
            p1p = np.zeros((128, p1cols), dtype=np.int16)
            p1p[:, :p1t.shape[1]] = p1t
            m[f"p1i{g}"] = p1p
            m[f"p2i{g}"] = p2t
        in_maps.append(m)

    return in_maps, tuple(caps), p1cols, p2cols, (rz_same, bhn_zero)


def _build_module(caps, p1cols, p2cols, flags):
    rz_same, bhn_zero = flags
    import concourse.mybir as mybir
    import concourse.tile as tile
    from concourse import bacc

    f32 = mybir.dt.float32
    bf16 = mybir.dt.bfloat16
    i16 = mybir.dt.int16
    Sig = mybir.ActivationFunctionType.Sigmoid
    Tanh = mybir.ActivationFunctionType.Tanh
    Alu = mybir.AluOpType

    R = math.ceil(N_NODES / NREG)

    nc = bacc.Bacc(None, target_bir_lowering=False, num_swdge_queues=4,
                   dynamic_dma_scratch_size=65536)

    xb_d = nc.dram_tensor("xb", [N_NODES, D], bf16, kind="ExternalInput")
    wcb_d = nc.dram_tensor("wcb", [128, 3 * H], bf16, kind="ExternalInput")
    whf_d = nc.dram_tensor("whf", [128, 3 * H], f32, kind="ExternalInput")
    b_d = nc.dram_tensor("bias", [128, 4], f32, kind="ExternalInput")
    p1_d = [nc.dram_tensor(f"p1i{g}", [128, p1cols], i16, kind="ExternalInput")
            for g in range(NPAIR)]
    p2_d = [nc.dram_tensor(f"p2i{g}", [128, p2cols], i16, kind="ExternalInput")
            for g in range(NPAIR)]
    out_d = nc.dram_tensor("out", [128, B], bf16, kind="ExternalOutput")

    # static phase-1 plan: per group, list of (region, idx_col_off, nidx, blk_off)
    p1_plan = []
    for capblk in caps:
        plan = []
        b0 = 0
        for r in range(NREG):
            cap = capblk[r] * 128
            off = 0
            while off < cap:
                n = min(P1_MAX, cap - off)
                plan.append((r, (b0 * 128 + off) // 16, n, b0 + off // 128))
                off += n
            b0 += capblk[r]
        p1_plan.append(plan)

    # phase-2 plan: chunks of the 2*B idx entries
    p2_chunks = []
    off = 0
    while off < 2 * B:
        k = min(P2_MAX, 2 * B - off)
        p2_chunks.append((off // 16, k, off))
        off += k

    Qn = [0]   # queue rotation state

    with tile.TileContext(nc) as tc:
        with tc.tile_pool(name="cst", bufs=1) as cst, \
             tc.tile_pool(name="sb", bufs=2) as sb, \
             tc.tile_pool(name="bkp", bufs=3) as bkp, \
             tc.tile_pool(name="xtp", bufs=4) as xtp, \
             tc.tile_pool(name="ps", bufs=1, space="PSUM") as ps:

            # ---- constants
            wtmp = cst.tile([128, 3 * H], f32, name="wtmp")
            nc.sync.dma_start(wtmp[:], whf_d[:])
            whr = cst.tile([128, 3 * H], bf16, name="whr")
            nc.vector.tensor_copy(whr[:], wtmp[:])
            wcb = cst.tile([128, 3 * H], bf16, name="wcb")
            nc.sync.dma_start(wcb[:], wcb_d[:])
            bias = cst.tile([128, 4], f32, name="bias")
            nc.sync.dma_start(bias[:], b_d[:])
            b_r = bias[:, 0:1]
            b_z = bias[:, 1:2]
            b_hn = bias[:, 2:3]
            b_xn = bias[:, 3:4]

            p1x = {}
            p2x = {}

            def load_idx(g):
                u1 = (sum(caps[g]) * 128) // 16
                p1x[g] = cst.tile([128, u1], i16, name=f"p1t{g}")
                nc.sync.dma_start(p1x[g][:], p1_d[g][:, 0:u1])
                p2x[g] = cst.tile([128, p2cols], i16, name=f"p2t{g}")
                nc.sync.dma_start(p2x[g][:], p2_d[g][:])

            def phase1_calls(g):
                bk = bkp.tile([128, sum(caps[g]), 128], bf16, tag="bkt",
                              name=f"bkt{g}")

                def emit(item):
                    (r, icol, n, boff) = item
                    rows = min(R, N_NODES - r * R)
                    nc.gpsimd.dma_gather(
                        out_ap=bk[:, boff:boff + n // 128, :],
                        in_ap=xb_d[r * R:r * R + rows, :],
                        idxs_ap=p1x[g][:, icol:icol + n // 16],
                        num_idxs=n, num_idxs_reg=n, elem_size=D,
                        queue_num=Qn[0] % 4,
                    )
                    Qn[0] += 1
                return bk, [lambda item=item: emit(item) for item in p1_plan[g]]

            def phase2_calls(g, bk):
                xT2 = xtp.tile([128, 2 * B], bf16, tag="xT", name=f"xT{g}")
                xT3 = xT2[:].rearrange("p (a b) -> p a b", a=1)

                def emit(item):
                    (icol, n, xoff) = item
                    nc.gpsimd.dma_gather(
                        out_ap=xT3[:, :, xoff:xoff + n],
                        in_ap=bk[:].rearrange("p a b -> p (a b)"),
                        idxs_ap=p2x[g][:, icol:icol + n // 16],
                        num_idxs=n, num_idxs_reg=n, elem_size=D,
                        transpose=True, queue_num=Qn[0] % 4,
                        sbuf_tokens_per_rank=128,
                        sbuf_free_dim_per_rank=256,
                        sbuf_free_dim_pad_per_rank=0,
                        sbuf_byte_offset=0,
                    )
                    Qn[0] += 1
                return xT2, [lambda item=item: emit(item) for item in p2_chunks]

            # ---- issue ALL gathers up-front: per pair, p1 then p2.
            # Pool executes in order; bkp/xtp pool rotation throttles
            # far-ahead calls automatically.
            # Issue order per pair g (v4-safe reuse distance, no Pool
            # head-of-line stall): all p1(g) calls were already emitted during
            # round g-1; here we interleave p2(g) chunks with p1(g+1) calls so
            # the Pool always has independent work queued while p2(g) waits
            # for p1(g)'s drains.
            xT_pend = {}
            load_idx(0)
            load_idx(1)
            bk_cur, p1_rem = phase1_calls(0)
            for f in p1_rem:
                f()
            for g in range(NPAIR):
                if g + 2 < NPAIR:
                    load_idx(g + 2)
                if g + 1 < NPAIR:
                    bk_next, p1_rem = phase1_calls(g + 1)
                else:
                    bk_next, p1_rem = None, []
                xT2, p2_rem = phase2_calls(g, bk_cur)
                # lead with two p1(g+1) calls to absorb p2(g)'s RAW wait,
                # then alternate.
                for f in p1_rem[:2]:
                    f()
                p1_rem = p1_rem[2:]
                while p2_rem or p1_rem:
                    if p2_rem:
                        p2_rem.pop(0)()
                    if p1_rem:
                        p1_rem.pop(0)()
                xT_pend[g] = xT2
                bk_cur = bk_next

            h0 = cst.tile([128, B], bf16, name="h0z")
            nc.vector.memset(h0[:], 0)
            h_prev = None
            for t in range(T):
                pair = t // 2
                loc = t % 2
                if loc == 0:
                    xT_cur = xT_pend.pop(pair)
                xT = xT_cur[:, loc * B:(loc + 1) * B]

                rz_h = {}
                narg_h = {}
                n_h = {}
                for hf in range(2):
                    rz_h[hf] = sb.tile([128, 2048], bf16, tag="rz",
                                       name=f"rz{t}_{hf}")
                    narg_h[hf] = sb.tile([128, HALF], bf16, tag="na",
                                         name=f"na{t}_{hf}")
                    n_h[hf] = sb.tile([128, HALF], bf16, tag="nn",
                                      name=f"nn{t}_{hf}")
                h_new = sb.tile([128, B], bf16, tag="h", name=f"h{t}")

                for q in range(4):
                    hf, qh = q // 2, q % 2
                    xTq = xT[:, q * Q:(q + 1) * Q]
                    RZ = ps.tile([128, 1024], f32, tag="big", bufs=4,
                                 name=f"RZ{t}_{q}")
                    NXH = ps.tile([128, 1024], f32, tag="big", bufs=4,
                                  name=f"NXH{t}_{q}")
                    first = t == 0
                    nc.tensor.matmul(out=RZ[:, 0:512], lhsT=wcb[:, 0:128],
                                     rhs=xTq, start=True, stop=first)
                    nc.tensor.matmul(out=RZ[:, 512:1024], lhsT=wcb[:, 128:256],
                                     rhs=xTq, start=True, stop=first)
                    nc.tensor.matmul(out=NXH[:, 0:512], lhsT=wcb[:, 256:384],
                                     rhs=xTq, start=True, stop=True)
                    if not first:
                        hq = h_prev[:, q * Q:(q + 1) * Q]
                        nc.tensor.matmul(out=RZ[:, 0:512], lhsT=whr[:, 0:128],
                                         rhs=hq, start=False, stop=True)
                        nc.tensor.matmul(out=RZ[:, 512:1024],
                                         lhsT=whr[:, 128:256],
                                         rhs=hq, start=False, stop=True)
                        nc.tensor.matmul(out=NXH[:, 512:1024],
                                         lhsT=whr[:, 256:384],
                                         rhs=hq, start=True, stop=True)

                    # sigmoid r|z -> rz_h[hf] at [r: qh*512, z: 1024+qh*512]
                    if rz_same:
                        out_ap = rz_h[hf][:].rearrange(
                            "p (g x) -> p g x", g=2)[:, :, qh * 512:(qh + 1) * 512]
                        nc.scalar.activation(out=out_ap, in_=RZ[:], func=Sig,
                                             bias=b_r)
                    else:
                        nc.scalar.activation(
                            out=rz_h[hf][:, qh * 512:(qh + 1) * 512],
                            in_=RZ[:, 0:512], func=Sig, bias=b_r)
                        nc.scalar.activation(
                            out=rz_h[hf][:, 1024 + qh * 512:1024 + (qh + 1) * 512],
                            in_=RZ[:, 512:1024], func=Sig, bias=b_z)

                    r_ap = rz_h[hf][:, qh * 512:(qh + 1) * 512]
                    nq = narg_h[hf][:, qh * 512:(qh + 1) * 512]
                    if first and bhn_zero:
                        nc.vector.tensor_copy(nq, NXH[:, 0:512])
                    else:
                        rhn = sb.tile([128, Q], bf16, tag="rhn", bufs=2,
                                      name=f"rhn{t}_{q}")
                        if not first:
                            if bhn_zero:
                                nc.vector.tensor_tensor(
                                    out=rhn[:], in0=NXH[:, 512:1024],
                                    in1=r_ap, op=Alu.mult)
                            else:
                                nc.vector.scalar_tensor_tensor(
                                    out=rhn[:], in0=NXH[:, 512:1024],
                                    scalar=b_hn, in1=r_ap,
                                    op0=Alu.add, op1=Alu.mult)
                        else:
                            nc.vector.tensor_scalar(
                                out=rhn[:], in0=r_ap, scalar1=b_hn,
                                scalar2=None, op0=Alu.mult)
                        nc.vector.tensor_tensor(
                            out=nq, in0=rhn[:], in1=NXH[:, 0:512], op=Alu.add)

                    # per-quarter tail: tanh + blend keep the last quarter's
                    # critical chain 512 wide instead of half-wide.
                    nn = n_h[hf][:, qh * 512:(qh + 1) * 512]
                    nc.scalar.activation(out=nn, in_=nq, func=Tanh, bias=b_xn)
                    d = sb.tile([128, Q], bf16, tag="d", bufs=2,
                                name=f"d{t}_{q}")
                    z_ap = rz_h[hf][:, 1024 + qh * 512:1024 + (qh + 1) * 512]
                    Sq = slice(q * Q, (q + 1) * Q)
                    hp = h_prev if t > 0 else h0
                    nc.vector.tensor_tensor(
                        out=d[:], in0=hp[:, Sq], in1=nn, op=Alu.subtract)
                    nc.vector.tensor_tensor(
                        out=d[:], in0=z_ap, in1=d[:], op=Alu.mult)
                    nc.vector.tensor_tensor(
                        out=h_new[:, Sq], in0=nn, in1=d[:], op=Alu.add)

                h_prev = h_new

            nc.sync.dma_start(out_d[:, 0:HALF], h_prev[:, 0:HALF])
            nc.sync.dma_start(out_d[:, HALF:B], h_prev[:, HALF:B])

    nc.compile()
    return nc


def _get_module(caps, p1cols, p2cols, flags):
    key = (caps, p1cols, p2cols, flags)
    if key not in _CACHE:
        _CACHE[key] = _build_module(caps, p1cols, p2cols, flags)
    return _CACHE[key]


def kernel(x, walks, W_f, b_f, W_ih, W_hh, b_ih, b_hh):
    from concourse.bass_utils import run_bass_kernel_spmd

    in_maps, caps, p1cols, p2cols, flags = _host_prep(
        x, walks, W_f, b_f, W_ih, W_hh, b_ih, b_hh)
    nc = _get_module(caps, p1cols, p2cols, flags)
    res = run_bass_kernel_spmd(nc, in_maps, core_ids=list(range(NCORES)))
    out = np.empty((B_TOTAL, H), dtype=np.float32)
    for c in range(NCORES):
        out[c * B:(c + 1) * B] = res.results[c]["out"].astype(np.float32).T
    return out


if __name__ == "__main__":
    rng = np.random.default_rng(0)
    ins = {
        "x": rng.standard_normal((N_NODES, D), dtype=np.float32),
        "walks": rng.integers(0, N_NODES, size=(B_TOTAL, T)).astype(np.int64),
        "W_f": rng.standard_normal((H, D), dtype=np.float32) / np.sqrt(D),
        "b_f": np.zeros(H, np.float32),
        "W_ih": rng.standard_normal((3 * H, H), dtype=np.float32) / np.sqrt(H),
        "W_hh": rng.standard_normal((3 * H, H), dtype=np.float32) / np.sqrt(H),
        "b_ih": np.zeros(3 * H, np.float32),
        "b_hh": np.zeros(3 * H, np.float32),
    }
    out = kernel(**ins)
    print(out.shape, out.dtype, float(np.abs(out).mean()))


# revision 33
# speedup vs baseline: 1.0141x; 1.0141x over previous
"""Trainium2 Bass kernel for ExplicitRandomWalkEncoder, v4.

Math:
    x_encoded = x @ W_f.T + b_f;  feats = x_encoded[walks]
    h_T = GRU(feats)  (torch gate order r,z,n)
Fold: gx = x[walks] @ (W_ih @ W_f).T + (W_ih b_f + b_ih), so the device
gathers raw x rows and applies W_c = W_ih @ W_f.

Sharding: data-parallel over walks; 2048 walks/core, x + weights replicated.

Gather engine: x is converted to bf16 on the host (256-byte rows). The
gather is drain-limited (~2.1 ns/descriptor across 4 SWDGE queues), so the
schedule interleaves phase-1 (HBM->bucket, deduped per 2-step group) and
phase-2 (bucket->xT transpose un-bucket) at step-pair granularity:

  for each pair p: p1(group p) then p2(pair p)

so delivery (~17 us/pair) stays ahead of consumption (~20 us/pair) with no
multi-call phase-1 blobs. Phase-1 index padding uses -1 (trailing negative
indices generate no descriptors, so padding costs no drain bandwidth).

GRU step (hidden-major, 4 quarter-chunks of 512 walks):
  PE   : r|z psum = Wc_rz @ xT (bf16) + Whh_rz @ h;  nx, nh psum
  ACT  : r,z = sigmoid(rz + b) -> bf16; n = tanh(narg + b_xn) -> bf16
  DVE  : rhn = (nh + b_hn) * r; narg = rhn + nx;
         d = h - n; d *= z; h' = n + d
"""

import math

import ml_dtypes
import numpy as np

N_NODES = 200000
D = 128
H = 128
B_TOTAL = 16384
T = 20
NCORES = 8
B = B_TOTAL // NCORES      # 2048 walks per core
Q = 512                    # quarter chunk
HALF = 1024
NPAIR = T // 2

NREG = 7                   # region count: ceil(N_NODES/NREG) < 32768
P1_MAX = 1024              # max descriptors per HBM dma_gather
P2_MAX = 896               # max descriptors per SBUF-source dma_gather

_CACHE = {}


def _roundup(x, m):
    return (x + m - 1) // m * m


def _wrap16(ids):
    """flat int list (len % 16 == 0) -> [128, len/16] int16 tile layout."""
    a = np.asarray(ids, dtype=np.int16).reshape(-1, 16).T   # [16, n/16]
    return np.tile(a, (8, 1))


def _host_prep(x, walks, W_f, b_f, W_ih, W_hh, b_ih, b_hh):
    x = np.asarray(x, dtype=np.float32)
    walks = np.asarray(walks).astype(np.int64)
    W_f = np.asarray(W_f, dtype=np.float32)
    b_f = np.asarray(b_f, dtype=np.float32)
    W_ih = np.asarray(W_ih, dtype=np.float32)
    W_hh = np.asarray(W_hh, dtype=np.float32)
    b_ih = np.asarray(b_ih, dtype=np.float32)
    b_hh = np.asarray(b_hh, dtype=np.float32)

    xb = np.ascontiguousarray(x).astype(ml_dtypes.bfloat16)

    W_c = (W_ih @ W_f).astype(np.float32)
    b_c = (W_ih @ b_f + b_ih).astype(np.float32)
    wcb = np.ascontiguousarray(W_c.T).astype(ml_dtypes.bfloat16)  # [128, 384]
    whf = np.ascontiguousarray(W_hh.T).astype(np.float32)         # [128, 384]
    bias = np.zeros((128, 4), dtype=np.float32)
    bias[:, 0] = b_c[0:128] + b_hh[0:128]       # b_r
    bias[:, 1] = b_c[128:256] + b_hh[128:256]   # b_z
    bias[:, 2] = b_hh[256:384]                  # b_hn
    bias[:, 3] = b_c[256:384]                   # b_xn
    rz_same = bool(np.allclose(bias[:, 0], bias[:, 1]))
    bhn_zero = bool(np.allclose(bias[:, 2], 0.0))

    # ---- per-core, per-pair bucketing (group g = steps 2g, 2g+1)
    # nodes[c][t][i] = walks[c*B + i, t]
    nodes = walks.reshape(NCORES, B, T).transpose(0, 2, 1)  # [C, T, B]
    R = math.ceil(N_NODES / NREG)

    caps = []       # per group: tuple of per-region capblk (uniform across cores)
    for g in range(NPAIR):
        t0, t1 = 2 * g, 2 * g + 2
        mx = np.zeros(NREG, dtype=np.int64)
        for c in range(NCORES):
            uniq = np.unique(nodes[c, t0:t1, :].ravel())
            cnt = np.bincount(np.minimum(uniq // R, NREG - 1), minlength=NREG)
            mx = np.maximum(mx, cnt)
        capr = [_roundup(max(int(v), 128), 128) for v in mx]
        assert max(capr) <= 1024, f"bucket overflow risk: {capr}"
        caps.append(tuple(cc // 128 for cc in capr))   # blocks per region

    p1cols = max((sum(cb) * 128) // 16 for cb in caps)
    p2cols = (2 * B) // 16

    in_maps = []
    for c in range(NCORES):
        m = {"xb": xb, "wcb": wcb, "whf": whf, "bias": bias}
        for g, capblk in enumerate(caps):
            t0 = 2 * g
            base = np.cumsum([0] + [cb * 128 for cb in capblk])
            p1 = [np.zeros(cb * 128, dtype=np.int16) for cb in capblk]
            cnt = np.zeros(NREG, dtype=np.int64)
            pos = np.zeros((2, B), dtype=np.int64)
            nd = nodes[c, t0:t0 + 2, :]
            rg = np.minimum(nd // R, NREG - 1)
            loc = nd - rg * R
            slot = {}
            for s in range(2):
                for i in range(B):
                    n_ = nd[s, i]
                    k = slot.get(n_)
                    if k is None:
                        r = rg[s, i]
                        k = base[r] + cnt[r]
                        p1[r][cnt[r]] = loc[s, i]
                        cnt[r] += 1
                        slot[n_] = k
                    pos[s, i] = k
            assert pos.max() < 32768
            p1t = _wrap16(np.concatenate(p1))
            p2t = _wrap16(pos.ravel().astype(np.int16))
            p1p = np.zeros((128, p1cols), dtype=np.int16)
            p1p[:, :p1t.shape[1]] = p1t
            m[f"p1i{g}"] = p1p
            m[f"p2i{g}"] = p2t
        in_maps.append(m)

    return in_maps, tuple(caps), p1cols, p2cols, (rz_same, bhn_zero)


def _build_module(caps, p1cols, p2cols, flags):
    rz_same, bhn_zero = flags
    import concourse.mybir as mybir
    import concourse.tile as tile
    from concourse import bacc

    f32 = mybir.dt.float32
    bf16 = mybir.dt.bfloat16
    i16 = mybir.dt.int16
    Sig = mybir.ActivationFunctionType.Sigmoid
    Tanh = mybir.ActivationFunctionType.Tanh
    Alu = mybir.AluOpType

    R = math.ceil(N_NODES / NREG)

    nc = bacc.Bacc(None, target_bir_lowering=False, num_swdge_queues=4,
                   dynamic_dma_scratch_size=65536)

    xb_d = nc.dram_tensor("xb", [N_NODES, D], bf16, kind="ExternalInput")
    wcb_d = nc.dram_tensor("wcb", [128, 3 * H], bf16, kind="ExternalInput")
    whf_d = nc.dram_tensor("whf", [128, 3 * H], f32, kind="ExternalInput")
    b_d = nc.dram_tensor("bias", [128, 4], f32, kind="ExternalInput")
    p1_d = [nc.dram_tensor(f"p1i{g}", [128, p1cols], i16, kind="ExternalInput")
            for g in range(NPAIR)]
    p2_d = [nc.dram_tensor(f"p2i{g}", [128, p2cols], i16, kind="ExternalInput")
            for g in range(NPAIR)]
    out_d = nc.dram_tensor("out", [128, B], bf16, kind="ExternalOutput")

    # static phase-1 plan: per group, list of (region, idx_col_off, nidx, blk_off)
    p1_plan = []
    for capblk in caps:
        plan = []
        b0 = 0
        for r in range(NREG):
            cap = capblk[r] * 128
            off = 0
            while off < cap:
                n = min(P1_MAX, cap - off)
                plan.append((r, (b0 * 128 + off) // 16, n, b0 + off // 128))
                off += n
            b0 += capblk[r]
        p1_plan.append(plan)

    # phase-2 plan: chunks of the 2*B idx entries
    p2_chunks = []
    off = 0
    while off < 2 * B:
        k = min(P2_MAX, 2 * B - off)
        p2_chunks.append((off // 16, k, off))
        off += k

    Qn = [0]   # queue rotation state

    with tile.TileContext(nc) as tc:
        with tc.tile_pool(name="cst", bufs=1) as cst, \
             tc.tile_pool(name="sb", bufs=2) as sb, \
             tc.tile_pool(name="bkp", bufs=3) as bkp, \
             tc.tile_pool(name="xtp", bufs=4) as xtp, \
             tc.tile_pool(name="ps", bufs=1, space="PSUM") as ps:

            # ---- gather-critical index DMAs first: the first p1 gather
            # waits only on p1i0, so it must be the first DMA in the queue
            # (weights aren't needed until the first matmul ~40 us later).
            p1x = {}
            p2x = {}

            def load_idx_early(g):
                u1 = (sum(caps[g]) * 128) // 16
                p1x[g] = cst.tile([128, u1], i16, name=f"p1t{g}")
                nc.sync.dma_start(p1x[g][:], p1_d[g][:, 0:u1])
                p2x[g] = cst.tile([128, p2cols], i16, name=f"p2t{g}")
                nc.sync.dma_start(p2x[g][:], p2_d[g][:])

            load_idx_early(0)
            load_idx_early(1)

            # ---- constants
            wtmp = cst.tile([128, 3 * H], f32, name="wtmp")
            nc.sync.dma_start(wtmp[:], whf_d[:])
            whr = cst.tile([128, 3 * H], bf16, name="whr")
            nc.vector.tensor_copy(whr[:], wtmp[:])
            wcb = cst.tile([128, 3 * H], bf16, name="wcb")
            nc.sync.dma_start(wcb[:], wcb_d[:])
            bias = cst.tile([128, 4], f32, name="bias")
            nc.sync.dma_start(bias[:], b_d[:])
            b_r = bias[:, 0:1]
            b_z = bias[:, 1:2]
            b_hn = bias[:, 2:3]
            b_xn = bias[:, 3:4]

            def load_idx(g):
                if g in p1x:
                    return
                u1 = (sum(caps[g]) * 128) // 16
                p1x[g] = cst.tile([128, u1], i16, name=f"p1t{g}")
                nc.sync.dma_start(p1x[g][:], p1_d[g][:, 0:u1])
                p2x[g] = cst.tile([128, p2cols], i16, name=f"p2t{g}")
                nc.sync.dma_start(p2x[g][:], p2_d[g][:])

            def phase1_calls(g):
                bk = bkp.tile([128, sum(caps[g]), 128], bf16, tag="bkt",
                              name=f"bkt{g}")

                def emit(item):
                    (r, icol, n, boff) = item
                    rows = min(R, N_NODES - r * R)
                    nc.gpsimd.dma_gather(
                        out_ap=bk[:, boff:boff + n // 128, :],
                        in_ap=xb_d[r * R:r * R + rows, :],
                        idxs_ap=p1x[g][:, icol:icol + n // 16],
                        num_idxs=n, num_idxs_reg=n, elem_size=D,
                        queue_num=Qn[0] % 4,
                    )
                    Qn[0] += 1
                return bk, [lambda item=item: emit(item) for item in p1_plan[g]]

            def phase2_calls(g, bk):
                xT2 = xtp.tile([128, 2 * B], bf16, tag="xT", name=f"xT{g}")
                xT3 = xT2[:].rearrange("p (a b) -> p a b", a=1)

                def emit(item):
                    (icol, n, xoff) = item
                    nc.gpsimd.dma_gather(
                        out_ap=xT3[:, :, xoff:xoff + n],
                        in_ap=bk[:].rearrange("p a b -> p (a b)"),
                        idxs_ap=p2x[g][:, icol:icol + n // 16],
                        num_idxs=n, num_idxs_reg=n, elem_size=D,
                        transpose=True, queue_num=Qn[0] % 4,
                        sbuf_tokens_per_rank=128,
                        sbuf_free_dim_per_rank=256,
                        sbuf_free_dim_pad_per_rank=0,
                        sbuf_byte_offset=0,
                    )
                    Qn[0] += 1
                return xT2, [lambda item=item: emit(item) for item in p2_chunks]

            # ---- issue ALL gathers up-front: per pair, p1 then p2.
            # Pool executes in order; bkp/xtp pool rotation throttles
            # far-ahead calls automatically.
            # Issue order per pair g (v4-safe reuse distance, no Pool
            # head-of-line stall): all p1(g) calls were already emitted during
            # round g-1; here we interleave p2(g) chunks with p1(g+1) calls so
            # the Pool always has independent work queued while p2(g) waits
            # for p1(g)'s drains.
            xT_pend = {}
            bk_cur, p1_rem = phase1_calls(0)
            for f in p1_rem:
                f()
            for g in range(NPAIR):
                if g + 2 < NPAIR:
                    load_idx(g + 2)
                if g + 1 < NPAIR:
                    bk_next, p1_rem = phase1_calls(g + 1)
                else:
                    bk_next, p1_rem = None, []
                xT2, p2_rem = phase2_calls(g, bk_cur)
                # lead with two p1(g+1) calls to absorb p2(g)'s RAW wait,
                # then alternate.
                for f in p1_rem[:2]:
                    f()
                p1_rem = p1_rem[2:]
                while p2_rem or p1_rem:
                    if p2_rem:
                        p2_rem.pop(0)()
                    if p1_rem:
                        p1_rem.pop(0)()
                xT_pend[g] = xT2
                bk_cur = bk_next

            h0 = cst.tile([128, B], bf16, name="h0z")
            nc.vector.memset(h0[:], 0)
            h_prev = None
            for t in range(T):
                pair = t // 2
                loc = t % 2
                if loc == 0:
                    xT_cur = xT_pend.pop(pair)
                xT = xT_cur[:, loc * B:(loc + 1) * B]

                rz_h = {}
                narg_h = {}
                n_h = {}
                for hf in range(2):
                    rz_h[hf] = sb.tile([128, 2048], bf16, tag="rz",
                                       name=f"rz{t}_{hf}")
                    narg_h[hf] = sb.tile([128, HALF], bf16, tag="na",
                                         name=f"na{t}_{hf}")
                    n_h[hf] = sb.tile([128, HALF], bf16, tag="nn",
                                      name=f"nn{t}_{hf}")
                h_new = sb.tile([128, B], bf16, tag="h", name=f"h{t}")

                for q in range(4):
                    hf, qh = q // 2, q % 2
                    xTq = xT[:, q * Q:(q + 1) * Q]
                    RZ = ps.tile([128, 1024], f32, tag="big", bufs=4,
                                 name=f"RZ{t}_{q}")
                    NXH = ps.tile([128, 1024], f32, tag="big", bufs=4,
                                  name=f"NXH{t}_{q}")
                    first = t == 0
                    nc.tensor.matmul(out=RZ[:, 0:512], lhsT=wcb[:, 0:128],
                                     rhs=xTq, start=True, stop=first)
                    nc.tensor.matmul(out=RZ[:, 512:1024], lhsT=wcb[:, 128:256],
                                     rhs=xTq, start=True, stop=first)
                    nc.tensor.matmul(out=NXH[:, 0:512], lhsT=wcb[:, 256:384],
                                     rhs=xTq, start=True, stop=True)
                    if not first:
                        hq = h_prev[:, q * Q:(q + 1) * Q]
                        nc.tensor.matmul(out=RZ[:, 0:512], lhsT=whr[:, 0:128],
                                         rhs=hq, start=False, stop=True)
                        nc.tensor.matmul(out=RZ[:, 512:1024],
                                         lhsT=whr[:, 128:256],
                                         rhs=hq, start=False, stop=True)
                        nc.tensor.matmul(out=NXH[:, 512:1024],
                                         lhsT=whr[:, 256:384],
                                         rhs=hq, start=True, stop=True)

                    # sigmoid r|z -> rz_h[hf] at [r: qh*512, z: 1024+qh*512]
                    if rz_same:
                        out_ap = rz_h[hf][:].rearrange(
                            "p (g x) -> p g x", g=2)[:, :, qh * 512:(qh + 1) * 512]
                        nc.scalar.activation(out=out_ap, in_=RZ[:], func=Sig,
                                             bias=b_r)
                    else:
                        nc.scalar.activation(
                            out=rz_h[hf][:, qh * 512:(qh + 1) * 512],
                            in_=RZ[:, 0:512], func=Sig, bias=b_r)
                        nc.scalar.activation(
                            out=rz_h[hf][:, 1024 + qh * 512:1024 + (qh + 1) * 512],
                            in_=RZ[:, 512:1024], func=Sig, bias=b_z)

                    r_ap = rz_h[hf][:, qh * 512:(qh + 1) * 512]
                    nq = narg_h[hf][:, qh * 512:(qh + 1) * 512]
                    if first and bhn_zero:
                        nc.vector.tensor_copy(nq, NXH[:, 0:512])
                    else:
                        rhn = sb.tile([128, Q], bf16, tag="rhn", bufs=2,
                                      name=f"rhn{t}_{q}")
                        if not first:
                            if bhn_zero:
                                nc.vector.tensor_tensor(
                                    out=rhn[:], in0=NXH[:, 512:1024],
                                    in1=r_ap, op=Alu.mult)
                            else:
                                nc.vector.scalar_tensor_tensor(
                                    out=rhn[:], in0=NXH[:, 512:1024],
                                    scalar=b_hn, in1=r_ap,
                                    op0=Alu.add, op1=Alu.mult)
                        else:
                            nc.vector.tensor_scalar(
                                out=rhn[:], in0=r_ap, scalar1=b_hn,
                                scalar2=None, op0=Alu.mult)
                        nc.vector.tensor_tensor(
                            out=nq, in0=rhn[:], in1=NXH[:, 0:512], op=Alu.add)

                    # per-quarter tail: tanh + blend keep the last quarter's
                    # critical chain 512 wide instead of half-wide.
                    nn = n_h[hf][:, qh * 512:(qh + 1) * 512]
                    nc.scalar.activation(out=nn, in_=nq, func=Tanh, bias=b_xn)
                    d = sb.tile([128, Q], bf16, tag="d", bufs=2,
                                name=f"d{t}_{q}")
                    z_ap = rz_h[hf][:, 1024 + qh * 512:1024 + (qh + 1) * 512]
                    Sq = slice(q * Q, (q + 1) * Q)
                    hp = h_prev if t > 0 else h0
                    nc.vector.tensor_tensor(
                        out=d[:], in0=hp[:, Sq], in1=nn, op=Alu.subtract)
                    nc.vector.tensor_tensor(
                        out=d[:], in0=z_ap, in1=d[:], op=Alu.mult)
                    nc.vector.tensor_tensor(
                        out=h_new[:, Sq], in0=nn, in1=d[:], op=Alu.add)

                h_prev = h_new

            nc.sync.dma_start(out_d[:, 0:HALF], h_prev[:, 0:HALF])
            nc.sync.dma_start(out_d[:, HALF:B], h_prev[:, HALF:B])

    nc.compile()
    return nc


def _get_module(caps, p1cols, p2cols, flags):
    key = (caps, p1cols, p2cols, flags)
    if key not in _CACHE:
        _CACHE[key] = _build_module(caps, p1cols, p2cols, flags)
    return _CACHE[key]


def kernel(x, walks, W_f, b_f, W_ih, W_hh, b_ih, b_hh):
    from concourse.bass_utils import run_bass_kernel_spmd

    in_maps, caps, p1cols, p2cols, flags = _host_prep(
        x, walks, W_f, b_f, W_ih, W_hh, b_ih, b_hh)
    nc = _get_module(caps, p1cols, p2cols, flags)
    res = run_bass_kernel_spmd(nc, in_maps, core_ids=list(range(NCORES)))
    out = np.empty((B_TOTAL, H), dtype=np.float32)
    for c in range(NCORES):
        out[c * B:(c + 1) * B] = res.results[c]["out"].astype(np.float32).T
    return out


if __name__ == "__main__":
    rng = np.random.default_rng(0)
    ins = {
        "x": rng.standard_normal((N_NODES, D), dtype=np.float32),
        "walks": rng.integers(0, N_NODES, size=(B_TOTAL, T)).astype(np.int64),
        "W_f": rng.standard_normal((H, D), dtype=np.float32) / np.sqrt(D),
        "b_f": np.zeros(H, np.float32),
        "W_ih": rng.standard_normal((3 * H, H), dtype=np.float32) / np.sqrt(H),
        "W_hh": rng.standard_normal((3 * H, H), dtype=np.float32) / np.sqrt(H),
        "b_ih": np.zeros(3 * H, np.float32),
        "b_hh": np.zeros(3 * H, np.float32),
    }
    out = kernel(**ins)
    print(out.shape, out.dtype, float(np.abs(out).mean()))
